# revision 9
# baseline (speedup 1.0000x reference)
"""GumbelVectorQuantizer forward on 8 trn2 NeuronCores (data-parallel).

Full inputs -> shard tokens across 8 cores -> bass/tile kernel per core ->
host finalization of tiny stats.

Per-core device work (Nc=4096 tokens, 32 tiles of 128):
  s = 10*x@E^T - 5*||e||^2           (PE fp32; bias via K=3 bf16 matmul)
  sg = s + gumbel                    (DVE)
  idx = argmax(sg)  (max8/max_index) -> indirect-DMA gather of embedding rows
  k   = argmax(s)   (via argmax of p = exp(s - max(sg)), monotone)
  p = exp(s - rowmax(sg)), Z = rowsum (ACT accum_out)
  probs_sum[m] += sum_t p[t,m]/Z[t]  (PE matmul with lhsT = 1/Z, float32r)
  sq_sums[:,i] = rowsum((x-q)^2)     (DVE sub + ACT Square accum_out)

Host: histogram(k) -> code_perplexity, probs_sum -> prob_perplexity,
      sq_sums -> commitment_loss, concat q -> quantized.
"""

import os
import sys
from contextlib import ExitStack

import numpy as np

sys.path.insert(0, "/opt/trn_rl_repo")

import ml_dtypes  # noqa: E402

B, T, D, M = 16, 2048, 512, 1024
N = B * T
NCORES = 8
NC_TOK = N // NCORES          # 4096 tokens per core
NTILES = NC_TOK // 128        # 32
TAU = 2.0
ALPHA = -5.0

LAST_RESULT = None  # BassKernelResults of the most recent run (for test.py)

_CACHED = {}


def _build_module():
    import concourse.bacc as bacc
    import concourse.tile as tile
    from concourse import mybir

    f32 = mybir.dt.float32
    f32r = mybir.dt.float32r
    bf16 = mybir.dt.bfloat16
    u32 = mybir.dt.uint32

    nc = bacc.Bacc(
        "TRN2",
        target_bir_lowering=False,
        debug=False,
        num_devices=NCORES,
    )

    # ---- per-core DRAM I/O ----
    xTh = nc.dram_tensor("xTh", [D, NC_TOK], bf16, kind="ExternalInput").ap()
    xTl = nc.dram_tensor("xTl", [D, NC_TOK], bf16, kind="ExternalInput").ap()
    xs = nc.dram_tensor("xs", [NC_TOK, D], f32, kind="ExternalInput").ap()
    g = nc.dram_tensor("g", [NC_TOK, M], f32, kind="ExternalInput").ap()
    eth = nc.dram_tensor("eth", [D, M], bf16, kind="ExternalInput").ap()
    etl = nc.dram_tensor("etl", [D, M], bf16, kind="ExternalInput").ap()
    bias3 = nc.dram_tensor("bias3", [3, M], bf16, kind="ExternalInput").ap()
    emb = nc.dram_tensor("emb", [M, D], f32, kind="ExternalInput").ap()

    q_out = nc.dram_tensor("q", [NC_TOK, D], f32, kind="ExternalOutput").ap()
    k_out = nc.dram_tensor("k", [NC_TOK, 1], u32, kind="ExternalOutput").ap()
    gi_out = nc.dram_tensor("gi", [NC_TOK, 1], u32, kind="ExternalOutput").ap()
    probs_out = nc.dram_tensor("probs", [2, 512], f32, kind="ExternalOutput").ap()
    sqs_out = nc.dram_tensor("sqs", [128, NTILES], f32, kind="ExternalOutput").ap()

    TOK_BLK = 512                  # tokens per xT DMA block (2KB lines)
    TILES_PER_BLK = TOK_BLK // 128

    with tile.TileContext(nc) as tc, ExitStack() as ctx:
        singles = ctx.enter_context(tc.tile_pool(name="singles", bufs=1))
        xt_pool = ctx.enter_context(tc.tile_pool(name="xt", bufs=3))
        g_pool = ctx.enter_context(tc.tile_pool(name="gp", bufs=4))
        xs_pool = ctx.enter_context(tc.tile_pool(name="xs", bufs=3))
        big_pool = ctx.enter_context(tc.tile_pool(name="big", bufs=4))
        q_pool = ctx.enter_context(tc.tile_pool(name="qp", bufs=4))
        small_pool = ctx.enter_context(tc.tile_pool(name="small", bufs=6))
        psum_s_pool = ctx.enter_context(
            tc.tile_pool(name="psum_s", bufs=3, space="PSUM")
        )
        psum_probs_pool = ctx.enter_context(
            tc.tile_pool(name="psum_probs", bufs=1, space="PSUM")
        )

        # resident constants
        eth_sb, etl_sb = [], []
        for c in range(4):
            th = singles.tile([128, M], bf16, tag=f"eth_{c}")
            nc.sync.dma_start(out=th[:], in_=eth[c * 128 : (c + 1) * 128, :])
            eth_sb.append(th)
            tl_ = singles.tile([128, M], bf16, tag=f"etl_{c}")
            nc.sync.dma_start(out=tl_[:], in_=etl[c * 128 : (c + 1) * 128, :])
            etl_sb.append(tl_)
        bias3_sb = singles.tile([3, M], bf16, tag="bias3")
        nc.sync.dma_start(out=bias3_sb[:], in_=bias3[:])
        ones3 = singles.tile([3, 128], bf16, tag="ones3")
        nc.vector.memset(ones3[:], 1.0)

        sq_sums = singles.tile([128, NTILES], f32, tag="sqsums")

        # persistent PSUM accumulators for probs sums (two banks)
        pr_a = psum_probs_pool.tile([1, 512], f32, tag="pra")
        pr_b = psum_probs_pool.tile([1, 512], f32, tag="prb")

        for blk in range(NTILES // TILES_PER_BLK):
            # xT block tiles: 4 chunks of [128 d, TOK_BLK]
            xth_sb, xtl_sb = [], []
            for c in range(4):
                th = xt_pool.tile([128, TOK_BLK], bf16, tag=f"xth_{c}")
                nc.sync.dma_start(
                    out=th[:],
                    in_=xTh[c * 128 : (c + 1) * 128,
                            blk * TOK_BLK : (blk + 1) * TOK_BLK],
                )
                xth_sb.append(th)
                tl_ = xt_pool.tile([128, TOK_BLK], bf16, tag=f"xtl_{c}")
                nc.sync.dma_start(
                    out=tl_[:],
                    in_=xTl[c * 128 : (c + 1) * 128,
                            blk * TOK_BLK : (blk + 1) * TOK_BLK],
                )
                xtl_sb.append(tl_)

            for j in range(TILES_PER_BLK):
                i = blk * TILES_PER_BLK + j
                r0 = i * 128
                first = i == 0
                last = i == NTILES - 1

                g_sb = g_pool.tile([128, M], f32, tag="gtile")
                nc.sync.dma_start(out=g_sb[:], in_=g[r0 : r0 + 128, :])
                xs_sb = xs_pool.tile([128, D], f32, tag="xstile")
                nc.sync.dma_start(out=xs_sb[:], in_=xs[r0 : r0 + 128, :])

                # ---- scores s = 10*x@E^T - 5*||e||^2 ----
                ps = psum_s_pool.tile([128, M], f32, tag="ps")
                tsl = slice(j * 128, (j + 1) * 128)
                for c in range(4):
                    for rhs_e in (eth_sb[c], etl_sb[c]):
                        for h in range(2):
                            nc.tensor.matmul(
                                ps[:, h * 512 : (h + 1) * 512],
                                lhsT=xth_sb[c][:, tsl],
                                rhs=rhs_e[:, h * 512 : (h + 1) * 512],
                                start=(c == 0 and rhs_e is eth_sb[c]),
                                stop=False,
                            )
                for c in range(4):
                    for h in range(2):
                        nc.tensor.matmul(
                            ps[:, h * 512 : (h + 1) * 512],
                            lhsT=xtl_sb[c][:, tsl],
                            rhs=eth_sb[c][:, h * 512 : (h + 1) * 512],
                            start=False,
                            stop=False,
                        )
                for h in range(2):
                    nc.tensor.matmul(
                        ps[:, h * 512 : (h + 1) * 512],
                        lhsT=ones3[:],
                        rhs=bias3_sb[:, h * 512 : (h + 1) * 512],
                        start=False,
                        stop=True,
                    )

                # ---- gumbel path ----
                sg = big_pool.tile([128, M], f32, tag="sg")
                nc.vector.tensor_add(sg[:], g_sb[:], ps[:])
                m8g = small_pool.tile([128, 8], f32, tag="m8g")
                nc.vector.max(out=m8g[:], in_=sg[:])
                mig = small_pool.tile([128, 8], u32, tag="mig")
                nc.vector.max_index(out=mig[:], in_max=m8g[:], in_values=sg[:])
                nc.sync.dma_start(out=gi_out[r0 : r0 + 128, :], in_=mig[:, 0:1])

                neg_rm2 = small_pool.tile([128, 1], f32, tag="negr")
                nc.vector.tensor_scalar_mul(neg_rm2[:], m8g[:, 0:1], -1.0)

                # ---- softmax (normalized by rowmax(sg), shift-invariant) ----
                p_sb = big_pool.tile([128, M], bf16, tag="ptile")
                zrow = small_pool.tile([128, 1], f32, tag="zrow")
                nc.scalar.activation(
                    out=p_sb[:],
                    in_=ps[:],
                    func=mybir.ActivationFunctionType.Exp,
                    bias=neg_rm2[:],
                    scale=1.0,
                    accum_out=zrow[:],
                )
                w = small_pool.tile([128, 1], bf16, tag="wrow")
                with nc.allow_low_precision(reason="w=1/Z bf16 for prob sums"):
                    nc.vector.reciprocal(out=w[:], in_=zrow[:])

                nc.tensor.matmul(
                    pr_a[:, :],
                    lhsT=w[:],
                    rhs=p_sb[:, 0:512],
                    start=first,
                    stop=last,
                    skip_group_check=True,
                )
                nc.tensor.matmul(
                    pr_b[:, :],
                    lhsT=w[:],
                    rhs=p_sb[:, 512:1024],
                    start=first,
                    stop=last,
                    skip_group_check=True,
                )

                # ---- hard argmax (argmax p == argmax s) ----
                m8p = small_pool.tile([128, 8], bf16, tag="m8p")
                nc.vector.max(out=m8p[:], in_=p_sb[:])
                kip = small_pool.tile([128, 8], u32, tag="kip")
                nc.vector.max_index(out=kip[:], in_max=m8p[:], in_values=p_sb[:])
                nc.sync.dma_start(out=k_out[r0 : r0 + 128, :], in_=kip[:, 0:1])

                # ---- gather quantized rows ----
                import concourse.bass as bass

                q_sb = q_pool.tile([128, D], f32, tag="qtile")
                nc.gpsimd.indirect_dma_start(
                    out=q_sb[:],
                    out_offset=None,
                    in_=emb[:],
                    in_offset=bass.IndirectOffsetOnAxis(ap=mig[:, 0:1], axis=0),
                )
                nc.sync.dma_start(out=q_out[r0 : r0 + 128, :], in_=q_sb[:])

                # ---- commitment partial: rowsum((x-q)^2) ----
                diff = q_pool.tile([128, D], f32, tag="dtile")
                nc.gpsimd.tensor_sub(diff[:], xs_sb[:], q_sb[:])
                sqsc = q_pool.tile([128, D], f32, tag="sqsc")
                nc.scalar.activation(
                    out=sqsc[:],
                    in_=diff[:],
                    func=mybir.ActivationFunctionType.Square,
                    bias=0.0,
                    scale=1.0,
                    accum_out=sq_sums[:, i : i + 1],
                )

        # ---- epilogue ----
        probs_sba = singles.tile([1, 512], f32, tag="probs_sba")
        probs_sbb = singles.tile([1, 512], f32, tag="probs_sbb")
        nc.scalar.copy(probs_sba[:], pr_a[:, :])
        nc.scalar.copy(probs_sbb[:], pr_b[:, :])
        nc.sync.dma_start(out=probs_out[0:1, :], in_=probs_sba[:])
        nc.sync.dma_start(out=probs_out[1:2, :], in_=probs_sbb[:])
        nc.sync.dma_start(out=sqs_out[:], in_=sq_sums[:])

    nc.compile()
    return nc


def _get_module():
    if "nc" not in _CACHED:
        _CACHED["nc"] = _build_module()
    return _CACHED["nc"]


def kernel(x, embedding, gumbel):
    global LAST_RESULT
    from concourse.bass_utils import run_bass_kernel_spmd

    x = np.asarray(x, dtype=np.float32)
    embedding = np.asarray(embedding, dtype=np.float32)
    gumbel = np.asarray(gumbel, dtype=np.float32)

    xf = x.reshape(N, D)

    # host prep of codebook-derived constants (O(M*D), tiny)
    et10 = np.ascontiguousarray((10.0 * embedding.astype(np.float64)).T).astype(
        np.float32
    )
    eth = et10.astype(ml_dtypes.bfloat16)
    etl = (et10 - eth.astype(np.float32)).astype(ml_dtypes.bfloat16)
    b = (-5.0 * (embedding.astype(np.float64) ** 2).sum(axis=1)).astype(np.float32)
    b0 = b.astype(ml_dtypes.bfloat16)
    r1 = (b - b0.astype(np.float32)).astype(np.float32)
    b1 = r1.astype(ml_dtypes.bfloat16)
    r2 = (r1 - b1.astype(np.float32)).astype(np.float32)
    b2 = r2.astype(ml_dtypes.bfloat16)
    bias3 = np.stack([b0, b1, b2], axis=0)  # [3, M] bf16

    # shard + transpose x
    xs_shards = [xf[c * NC_TOK : (c + 1) * NC_TOK] for c in range(NCORES)]
    xT_all = np.ascontiguousarray(
        xf.reshape(NCORES, NC_TOK, D).transpose(0, 2, 1)
    )
    xTh_all = xT_all.astype(ml_dtypes.bfloat16)
    xTl_all = (xT_all - xTh_all.astype(np.float32)).astype(ml_dtypes.bfloat16)
    g_shards = [gumbel[c * NC_TOK : (c + 1) * NC_TOK] for c in range(NCORES)]

    nc = _get_module()
    in_maps = [
        {
            "xTh": xTh_all[c],
            "xTl": xTl_all[c],
            "xs": np.ascontiguousarray(xs_shards[c]),
            "g": np.ascontiguousarray(g_shards[c]),
            "eth": eth,
            "etl": etl,
            "bias3": bias3,
            "emb": embedding,
        }
        for c in range(NCORES)
    ]

    res = run_bass_kernel_spmd(nc, in_maps, list(range(NCORES)))
    LAST_RESULT = res
    outs = res.results

    quantized = np.concatenate([outs[c]["q"] for c in range(NCORES)], axis=0)
    quantized = quantized.reshape(B, T, D)

    k_all = np.concatenate([outs[c]["k"][:, 0] for c in range(NCORES)])
    counts = np.bincount(k_all.astype(np.int64), minlength=M).astype(np.float32)
    hard_probs = counts / np.float32(N)
    code_perplexity = -np.sum(
        hard_probs * np.log2(hard_probs + np.float32(1e-10), dtype=np.float32)
    ).astype(np.float32)

    probs_total = np.zeros(M, dtype=np.float64)
    for c in range(NCORES):
        probs_total += outs[c]["probs"].reshape(M).astype(np.float64)
    avg_probs = (probs_total / N).astype(np.float32)
    prob_perplexity = -np.sum(
        avg_probs * np.log2(avg_probs + np.float32(1e-10), dtype=np.float32)
    ).astype(np.float32)

    sq_total = sum(float(outs[c]["sqs"].astype(np.float64).sum()) for c in range(NCORES))
    commitment_loss = np.float32(sq_total / (N * D))

    return (
        quantized,
        np.float32(code_perplexity),
        np.float32(prob_perplexity),
        commitment_loss,
    )


# revision 11
# speedup vs baseline: 1.0024x; 1.0024x over previous
"""GumbelVectorQuantizer forward on 8 trn2 NeuronCores (data-parallel).

Full inputs -> shard tokens across 8 cores -> bass/tile kernel per core ->
host finalization of tiny stats.

Per-core device work (Nc=4096 tokens, 32 tiles of 128):
  s = 10*x@E^T - 5*||e||^2           (PE fp32; bias via K=3 bf16 matmul)
  sg = s + gumbel                    (DVE)
  idx = argmax(sg)  (max8/max_index) -> indirect-DMA gather of embedding rows
  k   = argmax(s)   (via argmax of p = exp(s - max(sg)), monotone)
  p = exp(s - rowmax(sg)), Z = rowsum (ACT accum_out)
  probs_sum[m] += sum_t p[t,m]/Z[t]  (PE matmul with lhsT = 1/Z, float32r)
  sq_sums[:,i] = rowsum((x-q)^2)     (DVE sub + ACT Square accum_out)

Host: histogram(k) -> code_perplexity, probs_sum -> prob_perplexity,
      sq_sums -> commitment_loss, concat q -> quantized.
"""

import os
import sys
from contextlib import ExitStack

import numpy as np

sys.path.insert(0, "/opt/trn_rl_repo")

import ml_dtypes  # noqa: E402

B, T, D, M = 16, 2048, 512, 1024
N = B * T
NCORES = 8
NC_TOK = N // NCORES          # 4096 tokens per core
NTILES = NC_TOK // 128        # 32
TAU = 2.0
ALPHA = -5.0

LAST_RESULT = None  # BassKernelResults of the most recent run (for test.py)

_CACHED = {}


def _build_module():
    import concourse.bacc as bacc
    import concourse.tile as tile
    from concourse import mybir

    f32 = mybir.dt.float32
    f32r = mybir.dt.float32r
    bf16 = mybir.dt.bfloat16
    u32 = mybir.dt.uint32

    nc = bacc.Bacc(
        "TRN2",
        target_bir_lowering=False,
        debug=False,
        num_devices=NCORES,
    )

    # ---- per-core DRAM I/O ----
    xTh = nc.dram_tensor("xTh", [D, NC_TOK], bf16, kind="ExternalInput").ap()
    xTl = nc.dram_tensor("xTl", [D, NC_TOK], bf16, kind="ExternalInput").ap()
    xs = nc.dram_tensor("xs", [NC_TOK, D], f32, kind="ExternalInput").ap()
    g = nc.dram_tensor("g", [NC_TOK, M], f32, kind="ExternalInput").ap()
    eth = nc.dram_tensor("eth", [D, M], bf16, kind="ExternalInput").ap()
    etl = nc.dram_tensor("etl", [D, M], bf16, kind="ExternalInput").ap()
    bias3 = nc.dram_tensor("bias3", [3, M], bf16, kind="ExternalInput").ap()
    emb = nc.dram_tensor("emb", [M, D], f32, kind="ExternalInput").ap()

    q_out = nc.dram_tensor("q", [NC_TOK, D], f32, kind="ExternalOutput").ap()
    k_out = nc.dram_tensor("k", [NC_TOK, 1], u32, kind="ExternalOutput").ap()
    probs_out = nc.dram_tensor("probs", [2, 512], f32, kind="ExternalOutput").ap()
    sqs_out = nc.dram_tensor("sqs", [128, NTILES], f32, kind="ExternalOutput").ap()

    TOK_BLK = 512                  # tokens per xT DMA block (2KB lines)
    TILES_PER_BLK = TOK_BLK // 128

    with tile.TileContext(nc) as tc, ExitStack() as ctx:
        singles = ctx.enter_context(tc.tile_pool(name="singles", bufs=1))
        xt_pool = ctx.enter_context(tc.tile_pool(name="xt", bufs=3))
        g_pool = ctx.enter_context(tc.tile_pool(name="gp", bufs=4))
        xs_pool = ctx.enter_context(tc.tile_pool(name="xs", bufs=3))
        big_pool = ctx.enter_context(tc.tile_pool(name="big", bufs=4))
        q_pool = ctx.enter_context(tc.tile_pool(name="qp", bufs=4))
        small_pool = ctx.enter_context(tc.tile_pool(name="small", bufs=6))
        psum_s_pool = ctx.enter_context(
            tc.tile_pool(name="psum_s", bufs=3, space="PSUM")
        )
        psum_probs_pool = ctx.enter_context(
            tc.tile_pool(name="psum_probs", bufs=1, space="PSUM")
        )

        # resident constants
        eth_sb, etl_sb = [], []
        for c in range(4):
            th = singles.tile([128, M], bf16, tag=f"eth_{c}")
            nc.sync.dma_start(out=th[:], in_=eth[c * 128 : (c + 1) * 128, :])
            eth_sb.append(th)
            tl_ = singles.tile([128, M], bf16, tag=f"etl_{c}")
            nc.sync.dma_start(out=tl_[:], in_=etl[c * 128 : (c + 1) * 128, :])
            etl_sb.append(tl_)
        bias3_sb = singles.tile([3, M], bf16, tag="bias3")
        nc.sync.dma_start(out=bias3_sb[:], in_=bias3[:])
        ones3 = singles.tile([3, 128], bf16, tag="ones3")
        nc.vector.memset(ones3[:], 1.0)

        sq_sums = singles.tile([128, NTILES], f32, tag="sqsums")

        # persistent PSUM accumulators for probs sums (two banks)
        pr_a = psum_probs_pool.tile([1, 512], f32, tag="pra")
        pr_b = psum_probs_pool.tile([1, 512], f32, tag="prb")

        for blk in range(NTILES // TILES_PER_BLK):
            # xT block tiles: 4 chunks of [128 d, TOK_BLK]
            xth_sb, xtl_sb = [], []
            for c in range(4):
                th = xt_pool.tile([128, TOK_BLK], bf16, tag=f"xth_{c}")
                nc.sync.dma_start(
                    out=th[:],
                    in_=xTh[c * 128 : (c + 1) * 128,
                            blk * TOK_BLK : (blk + 1) * TOK_BLK],
                )
                xth_sb.append(th)
                tl_ = xt_pool.tile([128, TOK_BLK], bf16, tag=f"xtl_{c}")
                nc.sync.dma_start(
                    out=tl_[:],
                    in_=xTl[c * 128 : (c + 1) * 128,
                            blk * TOK_BLK : (blk + 1) * TOK_BLK],
                )
                xtl_sb.append(tl_)

            for j in range(TILES_PER_BLK):
                i = blk * TILES_PER_BLK + j
                r0 = i * 128
                first = i == 0
                last = i == NTILES - 1

                g_sb = g_pool.tile([128, M], f32, tag="gtile")
                nc.sync.dma_start(out=g_sb[:], in_=g[r0 : r0 + 128, :])
                xs_sb = xs_pool.tile([128, D], f32, tag="xstile")
                nc.sync.dma_start(out=xs_sb[:], in_=xs[r0 : r0 + 128, :])

                # ---- scores s = 10*x@E^T - 5*||e||^2 ----
                ps = psum_s_pool.tile([128, M], f32, tag="ps")
                tsl = slice(j * 128, (j + 1) * 128)
                for c in range(4):
                    for rhs_e in (eth_sb[c], etl_sb[c]):
                        for h in range(2):
                            nc.tensor.matmul(
                                ps[:, h * 512 : (h + 1) * 512],
                                lhsT=xth_sb[c][:, tsl],
                                rhs=rhs_e[:, h * 512 : (h + 1) * 512],
                                start=(c == 0 and rhs_e is eth_sb[c]),
                                stop=False,
                            )
                for c in range(4):
                    for h in range(2):
                        nc.tensor.matmul(
                            ps[:, h * 512 : (h + 1) * 512],
                            lhsT=xtl_sb[c][:, tsl],
                            rhs=eth_sb[c][:, h * 512 : (h + 1) * 512],
                            start=False,
                            stop=False,
                        )
                for h in range(2):
                    nc.tensor.matmul(
                        ps[:, h * 512 : (h + 1) * 512],
                        lhsT=ones3[:],
                        rhs=bias3_sb[:, h * 512 : (h + 1) * 512],
                        start=False,
                        stop=True,
                    )

                # ---- gumbel path ----
                sg = big_pool.tile([128, M], f32, tag="sg")
                nc.vector.tensor_add(sg[:], g_sb[:], ps[:])
                m8g = small_pool.tile([128, 8], f32, tag="m8g")
                nc.vector.max(out=m8g[:], in_=sg[:])
                mig = small_pool.tile([128, 8], u32, tag="mig")
                nc.vector.max_index(out=mig[:], in_max=m8g[:], in_values=sg[:])

                neg_rm2 = small_pool.tile([128, 1], f32, tag="negr")
                nc.vector.tensor_scalar_mul(neg_rm2[:], m8g[:, 0:1], -1.0)

                # ---- softmax (normalized by rowmax(sg), shift-invariant) ----
                p_sb = big_pool.tile([128, M], bf16, tag="ptile")
                zrow = small_pool.tile([128, 1], f32, tag="zrow")
                nc.scalar.activation(
                    out=p_sb[:],
                    in_=ps[:],
                    func=mybir.ActivationFunctionType.Exp,
                    bias=neg_rm2[:],
                    scale=1.0,
                    accum_out=zrow[:],
                )
                w = small_pool.tile([128, 1], bf16, tag="wrow")
                with nc.allow_low_precision(reason="w=1/Z bf16 for prob sums"):
                    nc.vector.reciprocal(out=w[:], in_=zrow[:])

                nc.tensor.matmul(
                    pr_a[:, :],
                    lhsT=w[:],
                    rhs=p_sb[:, 0:512],
                    start=first,
                    stop=last,
                    skip_group_check=True,
                )
                nc.tensor.matmul(
                    pr_b[:, :],
                    lhsT=w[:],
                    rhs=p_sb[:, 512:1024],
                    start=first,
                    stop=last,
                    skip_group_check=True,
                )

                # ---- hard argmax (argmax p == argmax s) ----
                m8p = small_pool.tile([128, 8], bf16, tag="m8p")
                nc.vector.max(out=m8p[:], in_=p_sb[:])
                kip = small_pool.tile([128, 8], u32, tag="kip")
                nc.vector.max_index(out=kip[:], in_max=m8p[:], in_values=p_sb[:])
                nc.sync.dma_start(out=k_out[r0 : r0 + 128, :], in_=kip[:, 0:1])

                # ---- gather quantized rows ----
                import concourse.bass as bass

                q_sb = q_pool.tile([128, D], f32, tag="qtile")
                nc.gpsimd.indirect_dma_start(
                    out=q_sb[:],
                    out_offset=None,
                    in_=emb[:],
                    in_offset=bass.IndirectOffsetOnAxis(ap=mig[:, 0:1], axis=0),
                )
                nc.sync.dma_start(out=q_out[r0 : r0 + 128, :], in_=q_sb[:])

                # ---- commitment partial: rowsum((x-q)^2) ----
                diff = q_pool.tile([128, D], f32, tag="dtile")
                nc.gpsimd.tensor_sub(diff[:], xs_sb[:], q_sb[:])
                sqsc = q_pool.tile([128, D], f32, tag="sqsc")
                nc.scalar.activation(
                    out=sqsc[:],
                    in_=diff[:],
                    func=mybir.ActivationFunctionType.Square,
                    bias=0.0,
                    scale=1.0,
                    accum_out=sq_sums[:, i : i + 1],
                )

        # ---- epilogue ----
        probs_sba = singles.tile([1, 512], f32, tag="probs_sba")
        probs_sbb = singles.tile([1, 512], f32, tag="probs_sbb")
        nc.scalar.copy(probs_sba[:], pr_a[:, :])
        nc.scalar.copy(probs_sbb[:], pr_b[:, :])
        nc.sync.dma_start(out=probs_out[0:1, :], in_=probs_sba[:])
        nc.sync.dma_start(out=probs_out[1:2, :], in_=probs_sbb[:])
        nc.sync.dma_start(out=sqs_out[:], in_=sq_sums[:])

    nc.compile()
    return nc


def _get_module():
    if "nc" not in _CACHED:
        _CACHED["nc"] = _build_module()
    return _CACHED["nc"]


def kernel(x, embedding, gumbel):
    global LAST_RESULT
    from concourse.bass_utils import run_bass_kernel_spmd

    x = np.asarray(x, dtype=np.float32)
    embedding = np.asarray(embedding, dtype=np.float32)
    gumbel = np.asarray(gumbel, dtype=np.float32)

    xf = x.reshape(N, D)

    # host prep of codebook-derived constants (O(M*D), tiny)
    et10 = np.ascontiguousarray((10.0 * embedding.astype(np.float64)).T).astype(
        np.float32
    )
    eth = et10.astype(ml_dtypes.bfloat16)
    etl = (et10 - eth.astype(np.float32)).astype(ml_dtypes.bfloat16)
    b = (-5.0 * (embedding.astype(np.float64) ** 2).sum(axis=1)).astype(np.float32)
    b0 = b.astype(ml_dtypes.bfloat16)
    r1 = (b - b0.astype(np.float32)).astype(np.float32)
    b1 = r1.astype(ml_dtypes.bfloat16)
    r2 = (r1 - b1.astype(np.float32)).astype(np.float32)
    b2 = r2.astype(ml_dtypes.bfloat16)
    bias3 = np.stack([b0, b1, b2], axis=0)  # [3, M] bf16

    # shard + transpose x
    xs_shards = [xf[c * NC_TOK : (c + 1) * NC_TOK] for c in range(NCORES)]
    xT_all = np.ascontiguousarray(
        xf.reshape(NCORES, NC_TOK, D).transpose(0, 2, 1)
    )
    xTh_all = xT_all.astype(ml_dtypes.bfloat16)
    xTl_all = (xT_all - xTh_all.astype(np.float32)).astype(ml_dtypes.bfloat16)
    g_shards = [gumbel[c * NC_TOK : (c + 1) * NC_TOK] for c in range(NCORES)]

    nc = _get_module()
    in_maps = [
        {
            "xTh": xTh_all[c],
            "xTl": xTl_all[c],
            "xs": np.ascontiguousarray(xs_shards[c]),
            "g": np.ascontiguousarray(g_shards[c]),
            "eth": eth,
            "etl": etl,
            "bias3": bias3,
            "emb": embedding,
        }
        for c in range(NCORES)
    ]

    res = run_bass_kernel_spmd(nc, in_maps, list(range(NCORES)))
    LAST_RESULT = res
    outs = res.results

    quantized = np.concatenate([outs[c]["q"] for c in range(NCORES)], axis=0)
    quantized = quantized.reshape(B, T, D)

    k_all = np.concatenate([outs[c]["k"][:, 0] for c in range(NCORES)])
    counts = np.bincount(k_all.astype(np.int64), minlength=M).astype(np.float32)
    hard_probs = counts / np.float32(N)
    code_perplexity = -np.sum(
        hard_probs * np.log2(hard_probs + np.float32(1e-10), dtype=np.float32)
    ).astype(np.float32)

    probs_total = np.zeros(M, dtype=np.float64)
    for c in range(NCORES):
        probs_total += outs[c]["probs"].reshape(M).astype(np.float64)
    avg_probs = (probs_total / N).astype(np.float32)
    prob_perplexity = -np.sum(
        avg_probs * np.log2(avg_probs + np.float32(1e-10), dtype=np.float32)
    ).astype(np.float32)

    sq_total = sum(float(outs[c]["sqs"].astype(np.float64).sum()) for c in range(NCORES))
    commitment_loss = np.float32(sq_total / (N * D))

    return (
        quantized,
        np.float32(code_perplexity),
        np.float32(prob_perplexity),
        commitment_loss,
    )
